# revision 26
# baseline (speedup 1.0000x reference)
"""Trainium2 Bass kernel for nn_Dilation2d (morphological max-plus dilation).

out[n,co,h,w] = max_{ci,kh,kw} x[n,ci,h+kh-2,w+kw-2] + weight[co,ci,kh,kw]
x: [8,4,512,512] f32, weight: [8,4,5,5] f32 -> out: [8,8,512,512] f32.

Strategy
--------
Data-parallel over batch N: one image per NeuronCore (8 cores).

Layout trick: W is split into 128 blocks of 4 columns; block p lives on SBUF
partition p. Each partition stores its 4 columns plus a +-2 column halo
(8 stored columns) for every padded row (516 rows, +-2 row halo) and every
input channel. With that layout BOTH the kh and kw shifts of the 5x5
structuring element are free-dimension offsets — no partition crossing, no
shifted copies, no halo exchange.

Default kernel (fp16, ~1.07 ms/core): per tap-round (ci,kh,kw), per-channel
adds tmp[o] = x_shift + w[o] run as fp16 tensor_scalar on the DVE (4x packed
mode; a second one-column-shifted input copy keeps every kw parity
4B-aligned) with the last N_ACT channels' adds offloaded to the scalar
engine (Identity activation with per-partition fp32 bias); then ONE merged
fp16 tensor_tensor max (2x mode, FD = 8*2048) folds all 8 channel planes
into the accumulator. Measured error vs the fp32 reference: absmax 4.3e-3
(5.2e-4 of output scale), max elementwise rel 1.1e-3, deterministic.

kernel_fp32 (bit-exact, ~1.77 ms/core): each tap is a fused fp32
scalar_tensor_tensor  acc = max(x_shifted + w_tap, acc)  on the DVE (no
fast mode exists for it, 1 elem/lane/cycle).

Everything fits in SBUF at once, so both kernels are raw Bass (no Tile)
with a handful of semaphores; loads/stores fully overlap compute (<2%).
"""

from contextlib import ExitStack

import numpy as np

import concourse.bass as bass
import concourse.mybir as mybir

# Problem constants (hardcoded; kernel.py must be self-contained).
N = 8
CI = 4
CO = 8
H = W = 512
K = 5  # Kh = Kw
PAD = K // 2  # 2
# Large finite negative instead of -inf: padding taps can never win (the
# center tap is always in-bounds), and finite values keep CoreSim's
# nonfinite-DMA check and HW fp32 arithmetic trouble-free.
NEG_INF = np.float32(-1e30)


def build_dilation_bass(h, w, ci=CI, co=CO, k=K, gps_co=0, reps=1):
    """Build the Bass program for one core's [ci,h,w] -> [co,h,w] dilation.

    Partitions carry w-blocks of 4 columns; free dim is (ci, padded row,
    stored col). The last `gps_co` output channels are computed by an
    ACT+GPSIMD pipeline (ACT: tmp = x + w via Identity-with-bias; GPSIMD:
    acc = max(acc, tmp) via tensor_tensor) running in parallel with the
    DVE's fused scalar_tensor_tensor taps. All fp32, bit-exact.

    NOTE: gps_co > 0 validates in CoreSim but does NOT compile with the
    bundled walrus build (it rejects every generic tensor op on the Pool
    engine: NCC_IXCG966). Kept for toolchains that support it; the
    shipped kernel() uses gps_co=0. `reps` repeats the tap program for
    wall-clock slope benchmarking (max is idempotent, result unchanged).
    """
    pad = k // 2
    blk = 4
    p_used = w // blk
    assert w % blk == 0 and p_used <= 128
    sw = blk + k - 1  # stored columns per partition (block + halo)
    hp = h + k - 1    # padded rows
    row_elems = ci * hp * sw
    co_d = co - gps_co  # channels on DVE

    nc = bass.Bass("TRN2")
    xin = nc.dram_tensor("xin", [p_used, row_elems], mybir.dt.float32,
                         kind="ExternalInput")
    wt = nc.dram_tensor("wt", [p_used, co * ci * k * k], mybir.dt.float32,
                        kind="ExternalInput")
    out = nc.dram_tensor("out", [co, p_used, h * blk], mybir.dt.float32,
                         kind="ExternalOutput")

    with ExitStack() as ctx:
        xt = ctx.enter_context(
            nc.sbuf_tensor("xt", [p_used, row_elems], mybir.dt.float32))
        wtt = ctx.enter_context(
            nc.sbuf_tensor("wtt", [p_used, co * ci * k * k], mybir.dt.float32))
        acc = ctx.enter_context(
            nc.sbuf_tensor("acc", [p_used, co * h * blk], mybir.dt.float32))
        # one semaphore per ci chunk: HWDGE DMAs on different queues finish
        # out of order, so a single counting sem can't express "chunk c done"
        ld_sems = [ctx.enter_context(nc.semaphore(f"ld_sem{c}"))
                   for c in range(ci)]
        chain = ctx.enter_context(nc.semaphore("chain"))
        if gps_co:
            tmp = ctx.enter_context(nc.sbuf_tensor(
                "tmp", [p_used, gps_co * 2 * h * blk], mybir.dt.float32))
            achain = ctx.enter_context(nc.semaphore("achain"))
            pchain = ctx.enter_context(nc.semaphore("pchain"))
        block = ctx.enter_context(nc.Block())

        # 4D views: xt as [p, ci, hp, sw]; acc as [p, co, h, blk]
        xt_v = xt.ap().rearrange("p (c r j) -> p c r j", c=ci, r=hp, j=sw)
        acc_v = acc.ap().rearrange("p (o r b) -> p o r b", o=co, r=h, b=blk)
        if gps_co:
            # tmp as [p, g, parity, h, blk] double-buffered per channel
            tmp_v = tmp.ap().rearrange(
                "p (g t r b) -> p g t r b", g=gps_co, t=2, r=h, b=blk)

        n_rounds = ci * k * k  # taps per output channel

        @block.sync
        def _(sync):
            # weights first (share chunk0's sem), then per-ci input chunks
            sync.dma_start(wtt[:, :], wt[:, :]).then_inc(ld_sems[0], 16)
            cl = hp * sw
            for c in range(ci):
                sync.dma_start(
                    xt[:, c * cl:(c + 1) * cl],
                    xin[:, c * cl:(c + 1) * cl],
                ).then_inc(ld_sems[c], 16)
            rr = reps * n_rounds  # total rounds including benchmark reps
            for o in range(co_d):
                # acc[o]'s last write is op (rr-1, o), the
                # (co_d*(rr-1) + o + 1)-th chain increment
                sync.wait_ge(chain, co_d * (rr - 1) + o + 1)
                sync.dma_start(
                    out[o, :, :],
                    acc[:, o * h * blk:(o + 1) * h * blk],
                ).then_inc(ld_sems[0], 16)
            for g in range(gps_co):
                o = co_d + g
                sync.wait_ge(pchain, gps_co * (rr - 1) + g + 1)
                sync.dma_start(
                    out[o, :, :],
                    acc[:, o * h * blk:(o + 1) * h * blk],
                ).then_inc(ld_sems[0], 16)

        def tap_program(eng, o_lo, o_hi, ch_sem):
            # Rounds over taps (c, kh, kw); within a round, channels
            # o_lo..o_hi-1. Consecutive ops hit different accumulators
            # (RAW distance o_hi-o_lo), and the one wait per round covers
            # every producer of the previous round, so the chain waits
            # never actually block.
            n_ch = o_hi - o_lo
            r = 0
            for rep in range(reps):
                for c in range(ci):
                    for kh in range(k):
                        for kw in range(k):
                            if rep == 0 and kh == 0 and kw == 0:
                                # weights (c==0) + input chunk c loaded
                                eng.wait_ge(ld_sems[c], 32 if c == 0 else 16)
                            if r > 0:
                                eng.wait_ge(ch_sem, n_ch * r)
                            in0 = xt_v[:, c, kh:kh + h, kw:kw + blk]
                            for o in range(o_lo, o_hi):
                                idx = ((o * ci + c) * k + kh) * k + kw
                                w_ap = wtt[:, idx:idx + 1]
                                a = acc_v[:, o, :, :]
                                if r % n_rounds == 0:
                                    # rep restart: plain add re-initializes
                                    ins = eng.tensor_scalar(
                                        a, in0, w_ap, None,
                                        mybir.AluOpType.add)
                                else:
                                    ins = eng.scalar_tensor_tensor(
                                        a, in0, w_ap, a,
                                        mybir.AluOpType.add,
                                        mybir.AluOpType.max)
                                ins.then_inc(ch_sem, 1)
                            r += 1

        @block.vector
        def _(vector):
            tap_program(vector, 0, co_d, chain)

        if gps_co:
            def rounds():
                r = 0
                for rep in range(reps):
                    for c in range(ci):
                        for kh in range(k):
                            for kw in range(k):
                                yield r, rep, c, kh, kw
                                r += 1

            @block.scalar
            def _(scalar):
                # ACT: tmp[g][r%2] = in0 + w (Identity with AP bias); runs
                # up to 2 rounds ahead of POOL (double-buffered tmp)
                for r, rep, c, kh, kw in rounds():
                    if rep == 0 and kh == 0 and kw == 0:
                        scalar.wait_ge(ld_sems[c], 32 if c == 0 else 16)
                    if r >= 2:
                        # tmp[g][r%2] last read by POOL round r-2
                        scalar.wait_ge(pchain, gps_co * (r - 1))
                    in0 = xt_v[:, c, kh:kh + h, kw:kw + blk]
                    for g in range(gps_co):
                        o = co_d + g
                        idx = ((o * ci + c) * k + kh) * k + kw
                        scalar.activation(
                            tmp_v[:, g, r % 2, :, :], in0,
                            mybir.ActivationFunctionType.Identity,
                            bias=wtt[:, idx:idx + 1], scale=1.0,
                        ).then_inc(achain, 1)

            @block.gpsimd
            def _(gpsimd):
                # POOL: acc[o] = max(acc[o], tmp[g][r%2]) via tensor_tensor
                for r, rep, c, kh, kw in rounds():
                    gpsimd.wait_ge(achain, gps_co * (r + 1))
                    if r > 0:
                        gpsimd.wait_ge(pchain, gps_co * r)
                    for g in range(gps_co):
                        o = co_d + g
                        a = acc_v[:, o, :, :]
                        t = tmp_v[:, g, r % 2, :, :]
                        if r % n_rounds == 0:
                            # (re)initialize: acc = max(tmp, tmp) = tmp
                            ins = gpsimd.tensor_tensor(
                                a, t, t, mybir.AluOpType.max)
                        else:
                            ins = gpsimd.tensor_tensor(
                                a, a, t, mybir.AluOpType.max)
                        ins.then_inc(pchain, 1)

    return nc


def build_dilation_bass_f16(h, w, ci=CI, co=CO, k=K, n_act=5, reps=1):
    """fp16 variant: per tap, tensor_scalar add (DVE 4x / ACT) into a tmp
    plane, then tensor_tensor max (DVE 2x_1P) into the fp16 accumulator.

    Two host-prepared copies of the halo-expanded input differ by one
    stored column (xina: j0=0, xinb: j0=1) so every tap's read is
    4-byte-aligned (even elem offset) regardless of kw parity -- the
    alignment requirement of the DVE 2x_1P/4x packed modes. The last
    `n_act` output channels' adds run on the scalar engine (ACT,
    Identity with per-partition fp32 bias) to offload the DVE.

    Weights stay fp32 (scalar operands are mode-exempt); all adds are
    computed in fp32 internally and rounded once to fp16. Measured error
    vs the fp32 reference: absmax ~4e-3 (5e-4 of scale), max
    elementwise rel ~1.1e-3.
    """
    blk = 4
    p_used = w // blk
    assert w % blk == 0 and p_used <= 128
    sw = blk + k - 1  # stored columns per partition (block + halo)
    hp = h + k - 1    # padded rows
    row_elems = ci * hp * sw
    n_dve = co - n_act  # channels whose adds run on DVE
    f16 = mybir.dt.float16

    nc = bass.Bass("TRN2")
    xina = nc.dram_tensor("xina", [p_used, row_elems], f16,
                          kind="ExternalInput")
    xinb = nc.dram_tensor("xinb", [p_used, row_elems], f16,
                          kind="ExternalInput")
    wt = nc.dram_tensor("wt", [p_used, co * ci * k * k], mybir.dt.float32,
                        kind="ExternalInput")
    out = nc.dram_tensor("out", [co, p_used, h * blk], f16,
                         kind="ExternalOutput")

    with ExitStack() as ctx:
        xta = ctx.enter_context(nc.sbuf_tensor("xta", [p_used, row_elems], f16))
        xtb = ctx.enter_context(nc.sbuf_tensor("xtb", [p_used, row_elems], f16))
        wtt = ctx.enter_context(nc.sbuf_tensor(
            "wtt", [p_used, co * ci * k * k], mybir.dt.float32))
        acc = ctx.enter_context(nc.sbuf_tensor("acc", [p_used, co * h * blk], f16))
        # tmp double-buffered (parity r%2), PARITY-MAJOR so one round's 8
        # channel planes are contiguous: the per-round max-merge is then a
        # single FD=co*h*blk tensor_tensor instead of co separate ones
        tmp = ctx.enter_context(nc.sbuf_tensor(
            "tmp", [p_used, 2 * co * h * blk], f16))
        ld_sems = [ctx.enter_context(nc.semaphore(f"ld_sem{c}"))
                   for c in range(ci)]
        chain = ctx.enter_context(nc.semaphore("chain"))
        achain = ctx.enter_context(nc.semaphore("achain"))
        block = ctx.enter_context(nc.Block())

        xta_v = xta.ap().rearrange("p (c r j) -> p c r j", c=ci, r=hp, j=sw)
        xtb_v = xtb.ap().rearrange("p (c r j) -> p c r j", c=ci, r=hp, j=sw)
        acc_v = acc.ap().rearrange("p (o r b) -> p o r b", o=co, r=h, b=blk)
        tmp_v = tmp.ap().rearrange("p (t o r b) -> p t o r b", t=2, o=co,
                                   r=h, b=blk)

        n_rounds = ci * k * k
        rr = reps * n_rounds
        # chain increments per round: n_dve TS + 1 merged TT
        per_round = n_dve + 1

        def in0_for(c, kh, kw):
            # even kw -> buffer A at offset kw; odd kw -> buffer B at kw-1
            # (B is shifted one column) so the elem offset is always even
            if kw % 2 == 0:
                return xta_v[:, c, kh:kh + h, kw:kw + blk]
            return xtb_v[:, c, kh:kh + h, kw - 1:kw - 1 + blk]

        def rounds():
            r = 0
            for rep in range(reps):
                for c in range(ci):
                    for kh in range(k):
                        for kw in range(k):
                            yield r, rep, c, kh, kw
                            r += 1

        @block.sync
        def _(sync):
            sync.dma_start(wtt[:, :], wt[:, :]).then_inc(ld_sems[0], 16)
            cl = hp * sw
            for c in range(ci):
                sync.dma_start(
                    xta[:, c * cl:(c + 1) * cl],
                    xina[:, c * cl:(c + 1) * cl]).then_inc(ld_sems[c], 16)
                sync.dma_start(
                    xtb[:, c * cl:(c + 1) * cl],
                    xinb[:, c * cl:(c + 1) * cl]).then_inc(ld_sems[c], 16)
            for o in range(co):
                # all acc planes last written by the merged TT of round rr-1
                sync.wait_ge(chain, per_round * rr)
                sync.dma_start(
                    out[o, :, :],
                    acc[:, o * h * blk:(o + 1) * h * blk],
                ).then_inc(ld_sems[0], 16)

        @block.scalar
        def _(scalar):
            # ACT: adds for the last n_act channels
            for r, rep, c, kh, kw in rounds():
                if n_act == 0:
                    break
                if rep == 0 and kh == 0 and kw == 0:
                    # both A/B copies of chunk c (+ weights for c==0)
                    scalar.wait_ge(ld_sems[c], 48 if c == 0 else 32)
                if r >= 2:
                    # tmp[o][r%2] last read by DVE TT block of round r-2
                    scalar.wait_ge(chain, per_round * (r - 1))
                in0 = in0_for(c, kh, kw)
                for o in range(n_dve, co):
                    idx = ((o * ci + c) * k + kh) * k + kw
                    scalar.activation(
                        tmp_v[:, r % 2, o, :, :], in0,
                        mybir.ActivationFunctionType.Identity,
                        bias=wtt[:, idx:idx + 1], scale=1.0,
                    ).then_inc(achain, 1)

        @block.vector
        def _(vector):
            for r, rep, c, kh, kw in rounds():
                if rep == 0 and kh == 0 and kw == 0:
                    vector.wait_ge(ld_sems[c], 48 if c == 0 else 32)
                if r > 0:
                    # TS block may overwrite tmp[o][r%2] read in round r-2
                    vector.wait_ge(chain, per_round * (r - 1))
                in0 = in0_for(c, kh, kw)
                for o in range(n_dve):
                    idx = ((o * ci + c) * k + kh) * k + kw
                    vector.tensor_scalar(
                        tmp_v[:, r % 2, o, :, :], in0, wtt[:, idx:idx + 1],
                        None, mybir.AluOpType.add).then_inc(chain, 1)
                # all tmps of this round ready (DVE's own + ACT's); the
                # chain wait is same-engine (never blocks) but gives the
                # race detector the TS(r) -> TT(r) edge
                vector.wait_ge(chain, per_round * r + n_dve)
                if n_act:
                    vector.wait_ge(achain, n_act * (r + 1))
                # single merged max over all co channel planes (FD=co*h*blk)
                a = acc[:, :]
                t = tmp_v[:, r % 2, :, :, :]
                if r % n_rounds == 0:
                    ins = vector.tensor_tensor(a, t, t, mybir.AluOpType.max)
                else:
                    ins = vector.tensor_tensor(a, a, t, mybir.AluOpType.max)
                ins.then_inc(chain, 1)

    return nc


def shard_inputs_f16(x, weight):
    """Host prep for the fp16 kernel: two kw-parity-aligned fp16 copies."""
    n, ci, h, w = x.shape
    pad = K // 2
    blk = 4
    p_used = w // blk
    sw = blk + K - 1
    hp = h + K - 1

    wt_flat = np.ascontiguousarray(weight.reshape(-1), dtype=np.float32)
    wt_host = np.ascontiguousarray(
        np.broadcast_to(wt_flat[None, :], (p_used, wt_flat.size)))

    in_maps = []
    for i in range(n):
        # one extra right pad column so the +1-shifted copy stays in range
        xpad = np.full((ci, hp, w + K), np.float16(-60000), dtype=np.float16)
        xpad[:, pad:pad + h, pad:pad + w] = x[i].astype(np.float16)
        s_ci, s_r, s_c = xpad.strides

        def expand(col0):
            v = np.lib.stride_tricks.as_strided(
                xpad[:, :, col0:], shape=(p_used, ci, hp, sw),
                strides=(blk * s_c, s_ci, s_r, s_c))
            return np.ascontiguousarray(v).reshape(p_used, ci * hp * sw)

        in_maps.append({"xina": expand(0), "xinb": expand(1),
                        "wt": wt_host})
    return in_maps


def shard_inputs(x, weight):
    """Host-side prep: per-core halo-expanded layout + broadcast weights."""
    n, ci, h, w = x.shape
    co, _, kh, kw = weight.shape
    assert (kh, kw) == (K, K)
    pad = K // 2
    blk = 4
    p_used = w // blk
    sw = blk + K - 1
    hp = h + K - 1

    wt_flat = np.ascontiguousarray(weight.reshape(-1), dtype=np.float32)
    wt_host = np.ascontiguousarray(
        np.broadcast_to(wt_flat[None, :], (p_used, wt_flat.size)))

    in_maps = []
    for i in range(n):
        xpad = np.full((ci, hp, w + K - 1), NEG_INF, dtype=np.float32)
        xpad[:, pad:pad + h, pad:pad + w] = x[i]
        s_ci, s_r, s_c = xpad.strides
        v = np.lib.stride_tricks.as_strided(
            xpad, shape=(p_used, ci, hp, sw),
            strides=(blk * s_c, s_ci, s_r, s_c))
        xin = np.ascontiguousarray(v).reshape(p_used, ci * hp * sw)
        in_maps.append({"xin": xin, "wt": wt_host})
    return in_maps


def unshard_output(results, h=H, w=W, co=CO):
    """results: list of per-core {'out': [co, p, h*4]} -> [n, co, h, w]."""
    blk = 4
    p_used = w // blk
    outs = []
    for r in results:
        o = r["out"].reshape(co, p_used, h, blk)
        outs.append(np.transpose(o, (0, 2, 1, 3)).reshape(co, h, w))
    return np.stack(outs, axis=0)


_CACHED = {}

# Number of output channels whose add stage runs on the scalar engine in
# the fp16 kernel (5 balances ACT vs DVE under the conservative 1-elem/
# cycle ACT assumption; it stays DVE-bound if ACT has a 2x fp16 mode).
N_ACT = 5


def unshard_output_f16(results, h=H, w=W, co=CO):
    blk = 4
    p_used = w // blk
    outs = []
    for r in results:
        o = r["out"].reshape(co, p_used, h, blk)
        outs.append(np.transpose(o, (0, 2, 1, 3)).reshape(co, h, w))
    return np.stack(outs, axis=0).astype(np.float32)


def kernel(x, weight):
    """fp16 pipeline (default): ~1.07 ms/core vs 1.77 ms for the exact
    fp32 path. Error vs the fp32 reference, measured on HW at full size:
    absmax 4.3e-3 (5.2e-4 of output scale), max elementwise rel 1.1e-3.
    Use kernel_fp32() for a bit-exact (0.0 error) result instead.
    """
    x = np.asarray(x, dtype=np.float32)
    weight = np.asarray(weight, dtype=np.float32)
    assert x.shape == (N, CI, H, W) and weight.shape == (CO, CI, K, K)

    from concourse.bass_utils import run_bass_kernel_spmd

    if "nc16" not in _CACHED:
        _CACHED["nc16"] = build_dilation_bass_f16(H, W, n_act=N_ACT)
    nc = _CACHED["nc16"]

    in_maps = shard_inputs_f16(x, weight)
    res = run_bass_kernel_spmd(nc, in_maps, core_ids=list(range(N)))
    return unshard_output_f16(res.results)


def kernel_fp32(x, weight):
    """Bit-exact fp32 path (fused scalar_tensor_tensor taps on DVE).

    Run in its own process: under the axon PJRT path, executing a second,
    different Bass program after kernel() in one process has returned
    corrupted results (runtime-level contamination; same-program repeat
    calls are fine and verified).
    """
    x = np.asarray(x, dtype=np.float32)
    weight = np.asarray(weight, dtype=np.float32)
    assert x.shape == (N, CI, H, W) and weight.shape == (CO, CI, K, K)

    from concourse.bass_utils import run_bass_kernel_spmd

    if "nc" not in _CACHED:
        _CACHED["nc"] = build_dilation_bass(H, W)
    nc = _CACHED["nc"]

    in_maps = shard_inputs(x, weight)
    res = run_bass_kernel_spmd(nc, in_maps, core_ids=list(range(N)))
    return unshard_output(res.results)


# revision 28
# speedup vs baseline: 1.0055x; 1.0055x over previous
"""Trainium2 Bass kernel for nn_Dilation2d (morphological max-plus dilation).

out[n,co,h,w] = max_{ci,kh,kw} x[n,ci,h+kh-2,w+kw-2] + weight[co,ci,kh,kw]
x: [8,4,512,512] f32, weight: [8,4,5,5] f32 -> out: [8,8,512,512] f32.

Strategy
--------
Data-parallel over batch N: one image per NeuronCore (8 cores).

Layout trick: W is split into 128 blocks of 4 columns; block p lives on SBUF
partition p. Each partition stores its 4 columns plus a +-2 column halo
(8 stored columns) for every padded row (516 rows, +-2 row halo) and every
input channel. With that layout BOTH the kh and kw shifts of the 5x5
structuring element are free-dimension offsets — no partition crossing, no
shifted copies, no halo exchange.

Default kernel (fp16, ~1.07 ms/core): per tap-round (ci,kh,kw), per-channel
adds tmp[o] = x_shift + w[o] run as fp16 tensor_scalar on the DVE (4x packed
mode; a second one-column-shifted input copy keeps every kw parity
4B-aligned) with the last N_ACT channels' adds offloaded to the scalar
engine (Identity activation with per-partition fp32 bias); then ONE merged
fp16 tensor_tensor max (2x mode, FD = 8*2048) folds all 8 channel planes
into the accumulator. Measured error vs the fp32 reference: absmax 4.3e-3
(5.2e-4 of output scale), max elementwise rel 1.1e-3, deterministic.

kernel_fp32 (bit-exact, ~1.77 ms/core): each tap is a fused fp32
scalar_tensor_tensor  acc = max(x_shifted + w_tap, acc)  on the DVE (no
fast mode exists for it, 1 elem/lane/cycle).

Everything fits in SBUF at once, so both kernels are raw Bass (no Tile)
with a handful of semaphores; loads/stores fully overlap compute (<2%).
"""

from contextlib import ExitStack

import numpy as np

import concourse.bass as bass
import concourse.mybir as mybir

# Problem constants (hardcoded; kernel.py must be self-contained).
N = 8
CI = 4
CO = 8
H = W = 512
K = 5  # Kh = Kw
PAD = K // 2  # 2
# Large finite negative instead of -inf: padding taps can never win (the
# center tap is always in-bounds), and finite values keep CoreSim's
# nonfinite-DMA check and HW fp32 arithmetic trouble-free.
NEG_INF = np.float32(-1e30)


def build_dilation_bass(h, w, ci=CI, co=CO, k=K, gps_co=0, reps=1):
    """Build the Bass program for one core's [ci,h,w] -> [co,h,w] dilation.

    Partitions carry w-blocks of 4 columns; free dim is (ci, padded row,
    stored col). The last `gps_co` output channels are computed by an
    ACT+GPSIMD pipeline (ACT: tmp = x + w via Identity-with-bias; GPSIMD:
    acc = max(acc, tmp) via tensor_tensor) running in parallel with the
    DVE's fused scalar_tensor_tensor taps. All fp32, bit-exact.

    NOTE: gps_co > 0 validates in CoreSim but does NOT compile with the
    bundled walrus build (it rejects every generic tensor op on the Pool
    engine: NCC_IXCG966). Kept for toolchains that support it; the
    shipped kernel() uses gps_co=0. `reps` repeats the tap program for
    wall-clock slope benchmarking (max is idempotent, result unchanged).
    """
    pad = k // 2
    blk = 4
    p_used = w // blk
    assert w % blk == 0 and p_used <= 128
    sw = blk + k - 1  # stored columns per partition (block + halo)
    hp = h + k - 1    # padded rows
    row_elems = ci * hp * sw
    co_d = co - gps_co  # channels on DVE

    nc = bass.Bass("TRN2")
    xin = nc.dram_tensor("xin", [p_used, row_elems], mybir.dt.float32,
                         kind="ExternalInput")
    wt = nc.dram_tensor("wt", [p_used, co * ci * k * k], mybir.dt.float32,
                        kind="ExternalInput")
    out = nc.dram_tensor("out", [co, p_used, h * blk], mybir.dt.float32,
                         kind="ExternalOutput")

    with ExitStack() as ctx:
        xt = ctx.enter_context(
            nc.sbuf_tensor("xt", [p_used, row_elems], mybir.dt.float32))
        wtt = ctx.enter_context(
            nc.sbuf_tensor("wtt", [p_used, co * ci * k * k], mybir.dt.float32))
        acc = ctx.enter_context(
            nc.sbuf_tensor("acc", [p_used, co * h * blk], mybir.dt.float32))
        # one semaphore per ci chunk: HWDGE DMAs on different queues finish
        # out of order, so a single counting sem can't express "chunk c done"
        ld_sems = [ctx.enter_context(nc.semaphore(f"ld_sem{c}"))
                   for c in range(ci)]
        chain = ctx.enter_context(nc.semaphore("chain"))
        if gps_co:
            tmp = ctx.enter_context(nc.sbuf_tensor(
                "tmp", [p_used, gps_co * 2 * h * blk], mybir.dt.float32))
            achain = ctx.enter_context(nc.semaphore("achain"))
            pchain = ctx.enter_context(nc.semaphore("pchain"))
        block = ctx.enter_context(nc.Block())

        # 4D views: xt as [p, ci, hp, sw]; acc as [p, co, h, blk]
        xt_v = xt.ap().rearrange("p (c r j) -> p c r j", c=ci, r=hp, j=sw)
        acc_v = acc.ap().rearrange("p (o r b) -> p o r b", o=co, r=h, b=blk)
        if gps_co:
            # tmp as [p, g, parity, h, blk] double-buffered per channel
            tmp_v = tmp.ap().rearrange(
                "p (g t r b) -> p g t r b", g=gps_co, t=2, r=h, b=blk)

        n_rounds = ci * k * k  # taps per output channel

        @block.sync
        def _(sync):
            # weights first (share chunk0's sem), then per-ci input chunks
            sync.dma_start(wtt[:, :], wt[:, :]).then_inc(ld_sems[0], 16)
            cl = hp * sw
            for c in range(ci):
                sync.dma_start(
                    xt[:, c * cl:(c + 1) * cl],
                    xin[:, c * cl:(c + 1) * cl],
                ).then_inc(ld_sems[c], 16)
            rr = reps * n_rounds  # total rounds including benchmark reps
            for o in range(co_d):
                # acc[o]'s last write is op (rr-1, o), the
                # (co_d*(rr-1) + o + 1)-th chain increment
                sync.wait_ge(chain, co_d * (rr - 1) + o + 1)
                sync.dma_start(
                    out[o, :, :],
                    acc[:, o * h * blk:(o + 1) * h * blk],
                ).then_inc(ld_sems[0], 16)
            for g in range(gps_co):
                o = co_d + g
                sync.wait_ge(pchain, gps_co * (rr - 1) + g + 1)
                sync.dma_start(
                    out[o, :, :],
                    acc[:, o * h * blk:(o + 1) * h * blk],
                ).then_inc(ld_sems[0], 16)

        def tap_program(eng, o_lo, o_hi, ch_sem):
            # Rounds over taps (c, kh, kw); within a round, channels
            # o_lo..o_hi-1. Consecutive ops hit different accumulators
            # (RAW distance o_hi-o_lo), and the one wait per round covers
            # every producer of the previous round, so the chain waits
            # never actually block.
            n_ch = o_hi - o_lo
            r = 0
            for rep in range(reps):
                for c in range(ci):
                    for kh in range(k):
                        for kw in range(k):
                            if rep == 0 and kh == 0 and kw == 0:
                                # weights (c==0) + input chunk c loaded
                                eng.wait_ge(ld_sems[c], 32 if c == 0 else 16)
                            if r > 0:
                                eng.wait_ge(ch_sem, n_ch * r)
                            in0 = xt_v[:, c, kh:kh + h, kw:kw + blk]
                            for o in range(o_lo, o_hi):
                                idx = ((o * ci + c) * k + kh) * k + kw
                                w_ap = wtt[:, idx:idx + 1]
                                a = acc_v[:, o, :, :]
                                if r % n_rounds == 0:
                                    # rep restart: plain add re-initializes
                                    ins = eng.tensor_scalar(
                                        a, in0, w_ap, None,
                                        mybir.AluOpType.add)
                                else:
                                    ins = eng.scalar_tensor_tensor(
                                        a, in0, w_ap, a,
                                        mybir.AluOpType.add,
                                        mybir.AluOpType.max)
                                ins.then_inc(ch_sem, 1)
                            r += 1

        @block.vector
        def _(vector):
            tap_program(vector, 0, co_d, chain)

        if gps_co:
            def rounds():
                r = 0
                for rep in range(reps):
                    for c in range(ci):
                        for kh in range(k):
                            for kw in range(k):
                                yield r, rep, c, kh, kw
                                r += 1

            @block.scalar
            def _(scalar):
                # ACT: tmp[g][r%2] = in0 + w (Identity with AP bias); runs
                # up to 2 rounds ahead of POOL (double-buffered tmp)
                for r, rep, c, kh, kw in rounds():
                    if rep == 0 and kh == 0 and kw == 0:
                        scalar.wait_ge(ld_sems[c], 32 if c == 0 else 16)
                    if r >= 2:
                        # tmp[g][r%2] last read by POOL round r-2
                        scalar.wait_ge(pchain, gps_co * (r - 1))
                    in0 = xt_v[:, c, kh:kh + h, kw:kw + blk]
                    for g in range(gps_co):
                        o = co_d + g
                        idx = ((o * ci + c) * k + kh) * k + kw
                        scalar.activation(
                            tmp_v[:, g, r % 2, :, :], in0,
                            mybir.ActivationFunctionType.Identity,
                            bias=wtt[:, idx:idx + 1], scale=1.0,
                        ).then_inc(achain, 1)

            @block.gpsimd
            def _(gpsimd):
                # POOL: acc[o] = max(acc[o], tmp[g][r%2]) via tensor_tensor
                for r, rep, c, kh, kw in rounds():
                    gpsimd.wait_ge(achain, gps_co * (r + 1))
                    if r > 0:
                        gpsimd.wait_ge(pchain, gps_co * r)
                    for g in range(gps_co):
                        o = co_d + g
                        a = acc_v[:, o, :, :]
                        t = tmp_v[:, g, r % 2, :, :]
                        if r % n_rounds == 0:
                            # (re)initialize: acc = max(tmp, tmp) = tmp
                            ins = gpsimd.tensor_tensor(
                                a, t, t, mybir.AluOpType.max)
                        else:
                            ins = gpsimd.tensor_tensor(
                                a, a, t, mybir.AluOpType.max)
                        ins.then_inc(pchain, 1)

    return nc


def build_dilation_bass_f16(h, w, ci=CI, co=CO, k=K, n_act=5, reps=1):
    """fp16 variant: per tap, tensor_scalar add (DVE 4x / ACT) into a tmp
    plane, then tensor_tensor max (DVE 2x_1P) into the fp16 accumulator.

    The host materializes all 5 kw shifts as DENSE copies (layout
    [ci][kw][row][4 cols] per partition), so every tap's read is one
    fully contiguous, 4B-aligned, even-length fp16 run -- textbook
    conditions for the DVE 4x packed mode, with no reliance on strided-AP
    mode detection. The last `n_act` output channels' adds run on the
    scalar engine (ACT, Identity with per-partition fp32 bias; 1 elem/
    cycle at any dtype per the engine docs) to offload the DVE.

    Weights stay fp32 (scalar operands are mode-exempt); all adds are
    computed in fp32 internally and rounded once to fp16. Measured error
    vs the fp32 reference: absmax ~4e-3 (5e-4 of scale), max
    elementwise rel ~1.1e-3.
    """
    blk = 4
    p_used = w // blk
    assert w % blk == 0 and p_used <= 128
    hp = h + k - 1    # padded rows
    row_elems = ci * k * hp * blk  # 5 dense kw-shifted copies per channel
    n_dve = co - n_act  # channels whose adds run on DVE
    f16 = mybir.dt.float16

    nc = bass.Bass("TRN2")
    xin5 = nc.dram_tensor("xin5", [p_used, row_elems], f16,
                          kind="ExternalInput")
    wt = nc.dram_tensor("wt", [p_used, co * ci * k * k], mybir.dt.float32,
                        kind="ExternalInput")
    out = nc.dram_tensor("out", [co, p_used, h * blk], f16,
                         kind="ExternalOutput")

    with ExitStack() as ctx:
        xt5 = ctx.enter_context(nc.sbuf_tensor("xt5", [p_used, row_elems], f16))
        wtt = ctx.enter_context(nc.sbuf_tensor(
            "wtt", [p_used, co * ci * k * k], mybir.dt.float32))
        acc = ctx.enter_context(nc.sbuf_tensor("acc", [p_used, co * h * blk], f16))
        # tmp double-buffered (parity r%2), PARITY-MAJOR so one round's 8
        # channel planes are contiguous: the per-round max-merge is then a
        # single FD=co*h*blk tensor_tensor instead of co separate ones
        tmp = ctx.enter_context(nc.sbuf_tensor(
            "tmp", [p_used, 2 * co * h * blk], f16))
        ld_sems = [ctx.enter_context(nc.semaphore(f"ld_sem{c}"))
                   for c in range(ci)]
        chain = ctx.enter_context(nc.semaphore("chain"))
        achain = ctx.enter_context(nc.semaphore("achain"))
        block = ctx.enter_context(nc.Block())

        xt5_v = xt5.ap().rearrange("p (c q r b) -> p c q r b", c=ci, q=k,
                                   r=hp, b=blk)
        acc_v = acc.ap().rearrange("p (o r b) -> p o r b", o=co, r=h, b=blk)
        tmp_v = tmp.ap().rearrange("p (t o r b) -> p t o r b", t=2, o=co,
                                   r=h, b=blk)

        n_rounds = ci * k * k
        rr = reps * n_rounds
        # chain increments per round: n_dve TS + 1 merged TT
        per_round = n_dve + 1

        def in0_for(c, kh, kw):
            # dense copy q=kw, rows kh..kh+h: one contiguous h*blk run
            return xt5_v[:, c, kw, kh:kh + h, :]

        def rounds():
            r = 0
            for rep in range(reps):
                for c in range(ci):
                    for kh in range(k):
                        for kw in range(k):
                            yield r, rep, c, kh, kw
                            r += 1

        @block.sync
        def _(sync):
            sync.dma_start(wtt[:, :], wt[:, :]).then_inc(ld_sems[0], 16)
            cl = k * hp * blk
            for c in range(ci):
                sync.dma_start(
                    xt5[:, c * cl:(c + 1) * cl],
                    xin5[:, c * cl:(c + 1) * cl]).then_inc(ld_sems[c], 16)
            for o in range(co):
                # the last round merges per-channel (not one big TT) so
                # each store can fire as soon as its plane is final
                sync.wait_ge(chain, per_round * (rr - 1) + n_dve + o + 1)
                sync.dma_start(
                    out[o, :, :],
                    acc[:, o * h * blk:(o + 1) * h * blk],
                ).then_inc(ld_sems[0], 16)

        @block.scalar
        def _(scalar):
            # ACT: adds for the last n_act channels
            for r, rep, c, kh, kw in rounds():
                if n_act == 0:
                    break
                if rep == 0 and kh == 0 and kw == 0:
                    # chunk c loaded (+ weights for c==0)
                    scalar.wait_ge(ld_sems[c], 32 if c == 0 else 16)
                if r >= 2:
                    # tmp[o][r%2] last read by DVE TT block of round r-2
                    scalar.wait_ge(chain, per_round * (r - 1))
                in0 = in0_for(c, kh, kw)
                for o in range(n_dve, co):
                    idx = ((o * ci + c) * k + kh) * k + kw
                    scalar.activation(
                        tmp_v[:, r % 2, o, :, :], in0,
                        mybir.ActivationFunctionType.Identity,
                        bias=wtt[:, idx:idx + 1], scale=1.0,
                    ).then_inc(achain, 1)

        @block.vector
        def _(vector):
            for r, rep, c, kh, kw in rounds():
                if rep == 0 and kh == 0 and kw == 0:
                    vector.wait_ge(ld_sems[c], 32 if c == 0 else 16)
                if r > 0:
                    # TS block may overwrite tmp[o][r%2] read in round r-2
                    vector.wait_ge(chain, per_round * (r - 1))
                in0 = in0_for(c, kh, kw)
                for o in range(n_dve):
                    idx = ((o * ci + c) * k + kh) * k + kw
                    vector.tensor_scalar(
                        tmp_v[:, r % 2, o, :, :], in0, wtt[:, idx:idx + 1],
                        None, mybir.AluOpType.add).then_inc(chain, 1)
                # all tmps of this round ready (DVE's own + ACT's); the
                # chain wait is same-engine (never blocks) but gives the
                # race detector the TS(r) -> TT(r) edge
                vector.wait_ge(chain, per_round * r + n_dve)
                if n_act:
                    vector.wait_ge(achain, n_act * (r + 1))
                if r == rr - 1:
                    # final round: per-channel merges so stores pipeline
                    for o in range(co):
                        a = acc_v[:, o, :, :]
                        t = tmp_v[:, r % 2, o, :, :]
                        vector.tensor_tensor(
                            a, a, t, mybir.AluOpType.max).then_inc(chain, 1)
                else:
                    # single merged max over all co planes (FD=co*h*blk)
                    a = acc[:, :]
                    t = tmp_v[:, r % 2, :, :, :]
                    if r % n_rounds == 0:
                        ins = vector.tensor_tensor(
                            a, t, t, mybir.AluOpType.max)
                    else:
                        ins = vector.tensor_tensor(
                            a, a, t, mybir.AluOpType.max)
                    ins.then_inc(chain, 1)

    return nc


def shard_inputs_f16(x, weight):
    """Host prep for the fp16 kernel: 5 dense kw-shifted fp16 copies,
    laid out [ci][kw][padded row][4 cols] per partition."""
    n, ci, h, w = x.shape
    pad = K // 2
    blk = 4
    p_used = w // blk
    hp = h + K - 1

    wt_flat = np.ascontiguousarray(weight.reshape(-1), dtype=np.float32)
    wt_host = np.ascontiguousarray(
        np.broadcast_to(wt_flat[None, :], (p_used, wt_flat.size)))

    in_maps = []
    for i in range(n):
        xpad = np.full((ci, hp, w + K - 1), np.float16(-60000),
                       dtype=np.float16)
        xpad[:, pad:pad + h, pad:pad + w] = x[i].astype(np.float16)
        s_ci, s_r, s_c = xpad.strides
        # [p, ci, kw, hp, blk]: partition p, copy kw holds cols 4p+kw..+3
        v = np.lib.stride_tricks.as_strided(
            xpad, shape=(p_used, ci, K, hp, blk),
            strides=(blk * s_c, s_ci, s_c, s_r, s_c))
        xin5 = np.ascontiguousarray(v).reshape(p_used, ci * K * hp * blk)
        in_maps.append({"xin5": xin5, "wt": wt_host})
    return in_maps


def shard_inputs(x, weight):
    """Host-side prep: per-core halo-expanded layout + broadcast weights."""
    n, ci, h, w = x.shape
    co, _, kh, kw = weight.shape
    assert (kh, kw) == (K, K)
    pad = K // 2
    blk = 4
    p_used = w // blk
    sw = blk + K - 1
    hp = h + K - 1

    wt_flat = np.ascontiguousarray(weight.reshape(-1), dtype=np.float32)
    wt_host = np.ascontiguousarray(
        np.broadcast_to(wt_flat[None, :], (p_used, wt_flat.size)))

    in_maps = []
    for i in range(n):
        xpad = np.full((ci, hp, w + K - 1), NEG_INF, dtype=np.float32)
        xpad[:, pad:pad + h, pad:pad + w] = x[i]
        s_ci, s_r, s_c = xpad.strides
        v = np.lib.stride_tricks.as_strided(
            xpad, shape=(p_used, ci, hp, sw),
            strides=(blk * s_c, s_ci, s_r, s_c))
        xin = np.ascontiguousarray(v).reshape(p_used, ci * hp * sw)
        in_maps.append({"xin": xin, "wt": wt_host})
    return in_maps


def unshard_output(results, h=H, w=W, co=CO):
    """results: list of per-core {'out': [co, p, h*4]} -> [n, co, h, w]."""
    blk = 4
    p_used = w // blk
    outs = []
    for r in results:
        o = r["out"].reshape(co, p_used, h, blk)
        outs.append(np.transpose(o, (0, 2, 1, 3)).reshape(co, h, w))
    return np.stack(outs, axis=0)


_CACHED = {}

# Number of output channels whose add stage runs on the scalar engine in
# the fp16 kernel (5 balances ACT vs DVE under the conservative 1-elem/
# cycle ACT assumption; it stays DVE-bound if ACT has a 2x fp16 mode).
N_ACT = 5


def unshard_output_f16(results, h=H, w=W, co=CO):
    blk = 4
    p_used = w // blk
    outs = []
    for r in results:
        o = r["out"].reshape(co, p_used, h, blk)
        outs.append(np.transpose(o, (0, 2, 1, 3)).reshape(co, h, w))
    return np.stack(outs, axis=0).astype(np.float32)


def kernel(x, weight):
    """fp16 pipeline (default): ~1.07 ms/core vs 1.77 ms for the exact
    fp32 path. Error vs the fp32 reference, measured on HW at full size:
    absmax 4.3e-3 (5.2e-4 of output scale), max elementwise rel 1.1e-3.
    Use kernel_fp32() for a bit-exact (0.0 error) result instead.
    """
    x = np.asarray(x, dtype=np.float32)
    weight = np.asarray(weight, dtype=np.float32)
    assert x.shape == (N, CI, H, W) and weight.shape == (CO, CI, K, K)

    from concourse.bass_utils import run_bass_kernel_spmd

    if "nc16" not in _CACHED:
        _CACHED["nc16"] = build_dilation_bass_f16(H, W, n_act=N_ACT)
    nc = _CACHED["nc16"]

    in_maps = shard_inputs_f16(x, weight)
    res = run_bass_kernel_spmd(nc, in_maps, core_ids=list(range(N)))
    return unshard_output_f16(res.results)


def kernel_fp32(x, weight):
    """Bit-exact fp32 path (fused scalar_tensor_tensor taps on DVE).

    Run in its own process: under the axon PJRT path, executing a second,
    different Bass program after kernel() in one process has returned
    corrupted results (runtime-level contamination; same-program repeat
    calls are fine and verified).
    """
    x = np.asarray(x, dtype=np.float32)
    weight = np.asarray(weight, dtype=np.float32)
    assert x.shape == (N, CI, H, W) and weight.shape == (CO, CI, K, K)

    from concourse.bass_utils import run_bass_kernel_spmd

    if "nc" not in _CACHED:
        _CACHED["nc"] = build_dilation_bass(H, W)
    nc = _CACHED["nc"]

    in_maps = shard_inputs(x, weight)
    res = run_bass_kernel_spmd(nc, in_maps, core_ids=list(range(N)))
    return unshard_output(res.results)
